# revision 1
# baseline (speedup 1.0000x reference)
"""AttentionGNNLayer Trainium2 kernel (8 NeuronCores, SPMD).

Math:  out = relu(segment_sum(h_proj[senders] * a[senders, receivers][:, None],
                              receivers, N))
with h_proj = h @ W, a = (h@Wq + bq) @ (h@Wk + bk)^T.

Sharding strategy: shard RECEIVER nodes across the 8 cores (1024 nodes each).
The edge list enters the kernel only through a per-core count matrix
Ct_c[m, n_loc] = #edges (m -> n_loc + 1024*c), built host-side while sharding
(pure index preprocessing). Per core, with n restricted to its 1024-node slice:

    k_c  = h_c @ Wk + bk                  (1024 x 256)   local slice only
    G    = Wq @ k_c^T                     (256 x 1024)   tiny
    A    = h @ G   (== q @ k_c^T, bq=0)   (8192 x 1024)
    S    = Ct_c * A                       (8192 x 1024)  sparse-masked logits
    P    = S^T @ h                        (1024 x 256)
    out_c = relu((P @ W)^T)               (256 x 1024)   == relu(S^T @ h_proj)^T

The two O(N*NL*D) matmuls (A and P) are the irreducible compute; everything
else is O(D^2*NL). All in bf16 with f32 PSUM accumulation; no collectives.
bq is asserted zero (the module spec fills it with zeros); bk is applied
exactly. Output is produced transposed and un-transposed on the host.
"""

import sys

sys.path.insert(0, "/opt/trn_rl_repo")
sys.path.insert(0, "/opt/pypackages")

import numpy as np
import ml_dtypes

N_NODES = 8192
D = 256
N_CORES = 8
NL = N_NODES // N_CORES  # 1024 receiver nodes per core
NCHUNK = N_NODES // 128  # 64 m-chunks of 128 rows

BF16 = ml_dtypes.bfloat16

_graph_cache = {}


def _build_graph():
    import concourse.bacc as bacc
    import concourse.mybir as mybir
    import concourse.tile as tile

    fp32 = mybir.dt.float32
    bf16 = mybir.dt.bfloat16
    int8 = mybir.dt.int8

    nc = bacc.Bacc("TRN2", target_bir_lowering=False, debug=False)

    hT_d = nc.declare_dram_parameter("hT", [2, 128, N_NODES], bf16, isOutput=False)
    hN_d = nc.declare_dram_parameter("hN", [NCHUNK, 128, D], bf16, isOutput=False)
    # packed per-f2-tile columns: [0:D]=WqkT, [D:2D]=W, [2D:2D+NL]=hTloc
    sm_d = nc.declare_dram_parameter("SM", [2, 128, NL + 2 * D], bf16, isOutput=False)
    g0_d = nc.declare_dram_parameter("g0", [2, 128, 1], fp32, isOutput=False)
    ct_d = nc.declare_dram_parameter("Ct", [NCHUNK, 128, NL], int8, isOutput=False)
    out_d = nc.declare_dram_parameter("out", [2, 128, NL], fp32, isOutput=True)

    Relu = mybir.ActivationFunctionType.Relu
    Identity = mybir.ActivationFunctionType.Identity
    Copy = mybir.ActivationFunctionType.Copy

    with tile.TileContext(nc) as tc:
        with (
            tc.tile_pool(name="big", bufs=1) as big,
            tc.tile_pool(name="ct", bufs=4) as ctp,
            tc.tile_pool(name="hn", bufs=4) as hnp,
            tc.tile_pool(name="s", bufs=3) as sp,
            tc.tile_pool(name="ppsum", bufs=2, space="PSUM") as ppsum,
            tc.tile_pool(name="apsum", bufs=2, space="PSUM") as apsum,
            tc.tile_pool(name="accpsum", bufs=1, space="PSUM") as accpsum,
        ):
            # ---- packed small inputs: one DMA per f2-tile ----
            SM = [
                big.tile([128, NL + 2 * D], bf16, tag=f"SM{t}", name=f"SM{t}")
                for t in range(2)
            ]
            g0t = [big.tile([128, 1], fp32, tag=f"g0{t}", name=f"g0{t}") for t in range(2)]
            for t in range(2):
                nc.sync.dma_start(SM[t][:, : 2 * D], sm_d[t, :, : 2 * D])
                nc.sync.dma_start(g0t[t][:], g0_d[t])
            for t in range(2):
                for half in range(2):
                    nc.sync.dma_start(
                        SM[t][:, 2 * D + half * 512 : 2 * D + (half + 1) * 512],
                        sm_d[t, :, 2 * D + half * 512 : 2 * D + (half + 1) * 512],
                    )

            # ---- PE warm-up: keep the HAM activity window busy during the
            # initial DMA wait so real matmuls start at 2.4 GHz ----
            wsrc = big.tile([128, 512], bf16, tag="wsrc", name="wsrc")
            nc.vector.memset(wsrc[:], 0.0)
            for wi in range(6):
                wps = ppsum.tile([128, 512], fp32, tag="proj")
                nc.tensor.matmul(
                    wps[:], wsrc[:, :128], wsrc[:], start=True, stop=True
                )

            # ---- full hT as 8 separate 1MB tiles per f-half (so each A
            # chunk depends only on its own DMA) ----
            hT = [
                [
                    big.tile([128, 1024], bf16, tag=f"hT{t}_{dc}", name=f"hT{t}_{dc}")
                    for dc in range(8)
                ]
                for t in range(2)
            ]
            for dc in range(8):
                for t in range(2):
                    nc.sync.dma_start(
                        hT[t][dc][:], hT_d[t, :, dc * 1024 : (dc + 1) * 1024]
                    )

            # ---- G = (Wq Wk^T) @ h_loc^T + (Wq bk) x 1  -> [2][128 f, NL] --
            G = [big.tile([128, NL], bf16, tag=f"G{t}", name=f"G{t}") for t in range(2)]
            for gf in range(2):
                for nk in range(NL // 512):
                    ps = ppsum.tile([128, 512], fp32, tag="proj")
                    for ft in range(2):
                        nc.tensor.matmul(
                            ps[:],
                            SM[ft][:, gf * 128 : (gf + 1) * 128],
                            SM[ft][:, 2 * D + nk * 512 : 2 * D + (nk + 1) * 512],
                            start=(ft == 0),
                            stop=(ft == 1),
                        )
                    nc.scalar.activation(
                        G[gf][:, nk * 512 : (nk + 1) * 512],
                        ps[:],
                        Identity,
                        bias=g0t[gf][:],
                    )

            # ---- main loop: A = h@G, S = Ct*A, P^T += hN^T @ S ----
            PT = [
                accpsum.tile([128, NL], fp32, tag=f"x{t}", name=f"PT{t}")
                for t in range(2)
            ]
            for c in range(NCHUNK):
                ctt = ctp.tile([128, NL], int8, tag="ct")
                nc.scalar.dma_start(ctt[:], ct_d[c])
                hnt = hnp.tile([128, D], bf16, tag="hn")
                nc.scalar.dma_start(hnt[:], hN_d[c])
                st = sp.tile([128, NL], bf16, tag="s")
                for nh in range(NL // 512):
                    aps = apsum.tile([128, 512], fp32, tag="a")
                    for ft in range(2):
                        nc.tensor.matmul(
                            aps[:],
                            hT[ft][c // 8][:, (c % 8) * 128 : (c % 8 + 1) * 128],
                            G[ft][:, nh * 512 : (nh + 1) * 512],
                            start=(ft == 0),
                            stop=(ft == 1),
                        )
                    nc.vector.tensor_mul(
                        st[:, nh * 512 : (nh + 1) * 512],
                        aps[:],
                        ctt[:, nh * 512 : (nh + 1) * 512],
                    )
                for fh in range(2):
                    for nh in range(NL // 512):
                        nc.tensor.matmul(
                            PT[fh][:, nh * 512 : (nh + 1) * 512],
                            hnt[:, fh * 128 : (fh + 1) * 128],
                            st[:, nh * 512 : (nh + 1) * 512],
                            start=(c == 0),
                            stop=(c == NCHUNK - 1),
                        )

            # ---- PT -> SBUF bf16, then aggT = W^T @ P^T ----
            PTs = [
                big.tile([128, NL], bf16, tag=f"PTs{t}", name=f"PTs{t}")
                for t in range(2)
            ]
            for fh in range(2):
                nc.scalar.activation(
                    PTs[fh][:, 0:512], PT[fh][:, 0:512], Copy
                )
                nc.vector.tensor_copy(
                    PTs[fh][:, 512:1024], PT[fh][:, 512:1024]
                )
            aggT = [
                accpsum.tile([128, NL], fp32, tag=f"x{t}", name=f"aggT{t}")
                for t in range(2)
            ]
            for dh in range(2):
                for nh in range(2):
                    for ft in range(2):
                        nc.tensor.matmul(
                            aggT[dh][:, nh * 512 : (nh + 1) * 512],
                            SM[ft][:, D + dh * 128 : D + (dh + 1) * 128],
                            PTs[ft][:, nh * 512 : (nh + 1) * 512],
                            start=(ft == 0),
                            stop=(ft == 1),
                        )

            # ---- relu + store (sliced so DMA overlaps relu) ----
            for fh in range(2):
                ot = big.tile([128, NL], fp32, tag=f"out{fh}", name=f"out{fh}")
                for sl in range(4):
                    if fh == 0:
                        nc.scalar.activation(
                            ot[:, sl * 256 : (sl + 1) * 256],
                            aggT[fh][:, sl * 256 : (sl + 1) * 256],
                            Relu,
                        )
                    else:
                        nc.vector.tensor_scalar_max(
                            ot[:, sl * 256 : (sl + 1) * 256],
                            aggT[fh][:, sl * 256 : (sl + 1) * 256],
                            0.0,
                        )
                    nc.sync.dma_start(
                        out_d[fh, :, sl * 256 : (sl + 1) * 256],
                        ot[:, sl * 256 : (sl + 1) * 256],
                    )

    nc.compile()
    return nc


def _get_graph():
    if "nc" not in _graph_cache:
        _graph_cache["nc"] = _build_graph()
    return _graph_cache["nc"]


def make_in_maps(h, W, Wq, bq, Wk, bk, senders, receivers):
    h = np.asarray(h, dtype=np.float32)
    W = np.asarray(W, dtype=np.float32)
    Wq = np.asarray(Wq, dtype=np.float32)
    Wk = np.asarray(Wk, dtype=np.float32)
    bq = np.asarray(bq, dtype=np.float32)
    bk = np.asarray(bk, dtype=np.float32)
    s = np.asarray(senders).astype(np.int64)
    r = np.asarray(receivers).astype(np.int64)

    # bq == 0 (module spec fills it with zeros) lets A = h @ (Wq @ k^T)
    # stand in exactly for q @ k^T.
    assert not np.any(bq), "kernel fast path assumes bq == 0"

    hT = np.ascontiguousarray(h.T).astype(BF16).reshape(2, 128, N_NODES)
    hN = h.astype(BF16).reshape(NCHUNK, 128, D)
    # folded attention weight product and bias (parameter preprocessing):
    # G = (Wq Wk^T) h_loc^T + (Wq bk) x 1^T  ==  q-free form of q @ k_c^T
    WqkT = (Wk @ Wq.T).astype(BF16).reshape(2, 128, D)
    g0 = (Wq @ bk).astype(np.float32).reshape(2, 128, 1)
    Wb = W.astype(BF16).reshape(2, 128, D)

    in_maps = []
    for c in range(N_CORES):
        lo = c * NL
        m = (r >= lo) & (r < lo + NL)
        idx = s[m] * NL + (r[m] - lo)
        Ct = np.bincount(idx, minlength=N_NODES * NL)
        assert Ct.max() < 128
        Ct = Ct.astype(np.int8).reshape(NCHUNK, 128, NL)
        hTloc = hT.reshape(2, 128, N_NODES)[:, :, lo : lo + NL]
        SMc = np.concatenate([WqkT, Wb, hTloc], axis=2)
        in_maps.append(
            {
                "hT": hT,
                "hN": hN,
                "SM": np.ascontiguousarray(SMc),
                "g0": g0,
                "Ct": Ct,
            }
        )
    return in_maps


def assemble_output(results):
    out = np.empty((N_NODES, D), np.float32)
    for c in range(N_CORES):
        aggT = np.asarray(results[c]["out"]).reshape(D, NL)
        out[c * NL : (c + 1) * NL] = aggT.T
    return out


def kernel(h, W, Wq, bq, Wk, bk, senders, receivers):
    from concourse.bass_utils import run_bass_kernel_spmd

    in_maps = make_in_maps(h, W, Wq, bq, Wk, bk, senders, receivers)
    nc = _get_graph()
    res = run_bass_kernel_spmd(nc, in_maps, list(range(N_CORES))).results
    return assemble_output(res)

